# revision 1
# baseline (speedup 1.0000x reference)
"""ChildSum TreeLSTM cell on 8 Trainium2 NeuronCores.

Data-parallel over the node axis N: each of the 8 cores processes N/8 nodes.
All feature dims live on SBUF partitions; nodes stream along the free dim.

Host-side prep (free wrt HW time): transpose x/h_msgs/c_msgs to
feature-major [feat, nodes] layouts, cast streams + weights to bf16,
pre-add bias pairs. Device computes, per 1024-node tile:
    wx     = x@Wf.T once, re-injected into each gate PSUM via identity matmul
    gates  f_k = sigmoid(wx + h_k@Uf.T + bf)            (PE + ACT)
    c_tild = sum_k f_k * c_k                            (DVE bf16 tree)
    h_tild = sum_k h_k                                  (DVE bf16 tree)
    iou    = x@Wiou.T + h_tild@Uiou.T + biou            (PE)
    i,o,u  = sigmoid/sigmoid/tanh                       (ACT)
    c = i*u + c_tild ; h = o*tanh(c)                    (DVE + ACT)
"""

import os

os.environ.setdefault("JAX_COMPILATION_CACHE_DIR", "/root/.cache/jax_bass")

import numpy as np
import ml_dtypes

import concourse.bass as bass
import concourse.mybir as mybir
import concourse.tile as tile
from concourse import bacc
from concourse.bass_utils import run_bass_kernel_spmd

BF16 = ml_dtypes.bfloat16
F32 = np.float32

N_CORES = 8
N_FULL = 65536
NSH = N_FULL // N_CORES  # nodes per core
H = 256
X_SIZE = 300
XP = 384  # x feature dim padded to 3*128
K = 4
TN = 1024  # nodes per on-chip tile

SIG = mybir.ActivationFunctionType.Sigmoid
TANH = mybir.ActivationFunctionType.Tanh

LAST_RESULTS = None  # BassKernelResults of the most recent run (for test harness)


def build_bass(nsh=NSH, tn=TN, repeat=1):
    f32 = mybir.dt.float32
    bf = mybir.dt.bfloat16
    nt = nsh // tn
    assert nsh % tn == 0

    nc = bacc.Bacc("TRN2", debug=False)
    nh = tn // 512  # matmul output must stay within one PSUM bank (512 fp32)

    def mm(out_ap, lhsT, rhs, start, stop):
        for s in range(nh):
            ssl = slice(s * 512, (s + 1) * 512)
            nc.tensor.matmul(out_ap[:, ssl], lhsT, rhs[:, ssl], start=start, stop=stop)

    xt = nc.dram_tensor("xt", [3, 128, nsh], bf, kind="ExternalInput")
    ht = nc.dram_tensor("ht", [K, 2, 128, nsh], bf, kind="ExternalInput")
    ct = nc.dram_tensor("ct", [K, 2, 128, nsh], bf, kind="ExternalInput")
    wf = nc.dram_tensor("wf", [3, 128, H], bf, kind="ExternalInput")
    uf = nc.dram_tensor("uf", [2, 128, H], bf, kind="ExternalInput")
    wiou = nc.dram_tensor("wiou", [3, 128, 3 * H], bf, kind="ExternalInput")
    uiou = nc.dram_tensor("uiou", [2, 128, 3 * H], bf, kind="ExternalInput")
    bfb = nc.dram_tensor("bfb", [2, 128], f32, kind="ExternalInput")
    biou = nc.dram_tensor("biou", [6, 128], f32, kind="ExternalInput")
    ident = nc.dram_tensor("ident", [128, 128], bf, kind="ExternalInput")
    # out[0] = h, out[1] = c; chunked [kind, hchunk, 128, nsh]; bf16, host upcasts
    out = nc.dram_tensor("out", [2, 2, 128, nsh], bf, kind="ExternalOutput")

    with tile.TileContext(nc) as tc:
        with (
            tc.tile_pool(name="consts", bufs=1) as consts,
            tc.tile_pool(name="xin", bufs=3) as xin,
            tc.tile_pool(name="hin", bufs=8) as hin,
            tc.tile_pool(name="cin", bufs=8) as cin,
            tc.tile_pool(name="work", bufs=2) as work,
            tc.tile_pool(name="fpool", bufs=4) as fpool,
            tc.tile_pool(name="ppool", bufs=5) as ppool,
            tc.tile_pool(name="accp", bufs=2) as accp,
            tc.tile_pool(name="outp", bufs=3) as outp,
            tc.tile_pool(name="pg", bufs=2, space="PSUM") as pgp,
            tc.tile_pool(name="pio", bufs=2, space="PSUM") as pioup,
        ):
            # gate-critical consts first so the first wx/gate matmuls start early
            wf_s = consts.tile([128, 3, H], bf)
            nc.sync.dma_start(wf_s[:], wf[:].rearrange("c p m -> p c m"))
            uf_s = consts.tile([128, 2, H], bf)
            nc.sync.dma_start(uf_s[:], uf[:].rearrange("c p m -> p c m"))
            id_s = consts.tile([128, 128], bf)
            nc.sync.dma_start(id_s[:], ident[:])
            bfb_s = consts.tile([128, 2], f32)
            nc.sync.dma_start(bfb_s[:], bfb[:].rearrange("c p -> p c"))
            wiou_s = consts.tile([128, 3, 3 * H], bf)
            nc.sync.dma_start(wiou_s[:], wiou[:].rearrange("c p m -> p c m"))
            uiou_s = consts.tile([128, 2, 3 * H], bf)
            nc.sync.dma_start(uiou_s[:], uiou[:].rearrange("c p m -> p c m"))
            biou_s = consts.tile([128, 6], f32)
            nc.sync.dma_start(biou_s[:], biou[:].rearrange("c p -> p c"))

            import contextlib

            rep_ctx = tc.For_i(0, repeat, 1) if repeat > 1 else contextlib.nullcontext()
            with rep_ctx:
              for t in range(nt):
                n0 = t * tn
                nsl = slice(n0, n0 + tn)

                xtile = xin.tile([128, 3, tn], bf, tag="x")
                nc.sync.dma_start(
                    xtile[:], xt[:, :, nsl].rearrange("c p n -> p c n")
                )
                htiles = []
                ctiles = []
                for k in range(K):
                    hk = hin.tile([128, 2, tn], bf, tag="h")
                    nc.sync.dma_start(
                        hk[:], ht[k, :, :, nsl].rearrange("c p n -> p c n")
                    )
                    htiles.append(hk)
                    ck = cin.tile([128, 2, tn], bf, tag="c")
                    nc.sync.dma_start(
                        ck[:], ct[k, :, :, nsl].rearrange("c p n -> p c n")
                    )
                    ctiles.append(ck)

                # h_tild per chunk (bf16 pairwise tree)
                htild = []
                for j in range(2):
                    t01 = work.tile([128, tn], bf, tag="t01")
                    nc.vector.tensor_add(
                        t01[:], htiles[0][:, j, :], htiles[1][:, j, :]
                    )
                    t23 = work.tile([128, tn], bf, tag="t23")
                    nc.vector.tensor_add(
                        t23[:], htiles[2][:, j, :], htiles[3][:, j, :]
                    )
                    hs = work.tile([128, tn], bf, tag="htild")
                    nc.vector.tensor_add(hs[:], t01[:], t23[:])
                    htild.append(hs)

                # wx = x@Wf.T once per chunk -> bf16 SBUF
                wx_sb = []
                for j in range(2):
                    jsl = slice(j * 128, (j + 1) * 128)
                    pwx = pioup.tile([128, tn], f32, tag="pio")
                    for xc in range(3):
                        mm(
                            pwx[:],
                            wf_s[:, xc, jsl],
                            xtile[:, xc, :],
                            start=(xc == 0),
                            stop=(xc == 2),
                        )
                    wxj = work.tile([128, tn], bf, tag="wx")
                    nc.vector.tensor_copy(wxj[:], pwx[:])
                    wx_sb.append(wxj)

                # forget gates + c_tild per chunk
                ctild = []
                for j in range(2):
                    jsl = slice(j * 128, (j + 1) * 128)
                    prods = []
                    for k in range(K):
                        pg = pgp.tile([128, tn], f32, tag="pg")
                        # inject wx via identity matmul, then accumulate uh
                        mm(pg[:], id_s[:], wx_sb[j][:], start=True, stop=False)
                        for hc in range(2):
                            mm(
                                pg[:],
                                uf_s[:, hc, jsl],
                                htiles[k][:, hc, :],
                                start=False,
                                stop=(hc == 1),
                            )
                        fk = fpool.tile([128, tn], bf, tag="f")
                        nc.scalar.activation(
                            fk[:], pg[:], SIG, bias=bfb_s[:, j : j + 1]
                        )
                        pk = ppool.tile([128, tn], bf, tag="p")
                        nc.vector.tensor_mul(pk[:], fk[:], ctiles[k][:, j, :])
                        prods.append(pk)
                    s01 = accp.tile([128, tn], bf, tag="s01")
                    nc.vector.tensor_add(s01[:], prods[0][:], prods[1][:])
                    s23 = accp.tile([128, tn], bf, tag="s23")
                    nc.vector.tensor_add(s23[:], prods[2][:], prods[3][:])
                    cs = accp.tile([128, tn], bf, tag="ctild")
                    nc.vector.tensor_add(cs[:], s01[:], s23[:])
                    ctild.append(cs)

                # iou + outputs per chunk
                for j in range(2):
                    iou_sb = {}
                    for name, oc, func in (
                        ("i", j, SIG),
                        ("o", 2 + j, SIG),
                        ("u", 4 + j, TANH),
                    ):
                        pio = pioup.tile([128, tn], f32, tag="pio")
                        osl = slice(oc * 128, (oc + 1) * 128)
                        for xc in range(3):
                            mm(
                                pio[:],
                                wiou_s[:, xc, osl],
                                xtile[:, xc, :],
                                start=(xc == 0),
                                stop=False,
                            )
                        for hc in range(2):
                            mm(
                                pio[:],
                                uiou_s[:, hc, osl],
                                htild[hc][:],
                                start=False,
                                stop=(hc == 1),
                            )
                        g = fpool.tile([128, tn], bf, tag="g" + name)
                        nc.scalar.activation(
                            g[:], pio[:], func, bias=biou_s[:, oc : oc + 1]
                        )
                        iou_sb[name] = g

                    ciu = outp.tile([128, tn], bf, tag="ciu")
                    nc.vector.tensor_mul(ciu[:], iou_sb["i"][:], iou_sb["u"][:])
                    c_j = outp.tile([128, tn], bf, tag="cout")
                    nc.vector.tensor_add(c_j[:], ciu[:], ctild[j][:])
                    tanh_c = outp.tile([128, tn], bf, tag="tanhc")
                    nc.scalar.activation(tanh_c[:], c_j[:], TANH)
                    h_j = outp.tile([128, tn], bf, tag="hout")
                    nc.vector.tensor_mul(h_j[:], iou_sb["o"][:], tanh_c[:])

                    nc.sync.dma_start(out[0, j, :, nsl], h_j[:])
                    nc.sync.dma_start(out[1, j, :, nsl], c_j[:])

    nc.compile()
    return nc


_NC_CACHE = {}


def _get_nc(nsh, tn):
    key = (nsh, tn)
    if key not in _NC_CACHE:
        _NC_CACHE[key] = build_bass(nsh, tn)
    return _NC_CACHE[key]


def prep_host_inputs(x, h_msgs, c_msgs, W_iou, b_iou, U_iou, b_Uiou, W_f, b_Wf, U_f, b_Uf):
    """Full-input -> per-core input maps (host-side layout only)."""
    n = x.shape[0]
    nsh = n // N_CORES

    xp = np.zeros((XP, n), F32)
    xp[:X_SIZE] = x.T
    xt_full = np.ascontiguousarray(xp).astype(BF16).reshape(3, 128, n)

    ht_full = np.ascontiguousarray(h_msgs.astype(BF16).transpose(1, 2, 0)).reshape(
        K, 2, 128, n
    )
    ct_full = np.ascontiguousarray(c_msgs.astype(BF16).transpose(1, 2, 0)).reshape(
        K, 2, 128, n
    )

    wfp = np.zeros((XP, H), F32)
    wfp[:X_SIZE] = W_f.T
    wf_host = wfp.astype(BF16).reshape(3, 128, H)
    uf_host = np.ascontiguousarray(U_f.T).astype(BF16).reshape(2, 128, H)
    wioup = np.zeros((XP, 3 * H), F32)
    wioup[:X_SIZE] = W_iou.T
    wiou_host = wioup.astype(BF16).reshape(3, 128, 3 * H)
    uiou_host = np.ascontiguousarray(U_iou.T).astype(BF16).reshape(2, 128, 3 * H)

    bfb_host = (b_Wf + b_Uf).astype(F32).reshape(2, 128)
    biou_host = (b_iou + b_Uiou).astype(F32).reshape(6, 128)
    ident_host = np.eye(128, dtype=F32).astype(BF16)

    in_maps = []
    for c in range(N_CORES):
        sl = slice(c * nsh, (c + 1) * nsh)
        in_maps.append(
            {
                "xt": np.ascontiguousarray(xt_full[:, :, sl]),
                "ht": np.ascontiguousarray(ht_full[:, :, :, sl]),
                "ct": np.ascontiguousarray(ct_full[:, :, :, sl]),
                "wf": wf_host,
                "uf": uf_host,
                "wiou": wiou_host,
                "uiou": uiou_host,
                "bfb": bfb_host,
                "biou": biou_host,
                "ident": ident_host,
            }
        )
    return in_maps


def kernel(**inputs):
    global LAST_RESULTS
    inputs = {k: np.asarray(v) for k, v in inputs.items()}
    n = inputs["x"].shape[0]
    assert n == N_FULL, f"hardcoded for N={N_FULL}, got {n}"
    nsh = n // N_CORES

    nc = _get_nc(nsh, TN)
    in_maps = prep_host_inputs(**inputs)

    res = None
    for attempt in range(3):
        try:
            res = run_bass_kernel_spmd(nc, in_maps, core_ids=list(range(N_CORES)))
            break
        except Exception:
            if attempt == 2:
                raise
            import time as _time

            _time.sleep(5.0)
    LAST_RESULTS = res

    # results[c]["out"]: [2, 2, 128, nsh] -> full [2, N, 256]
    per_core = [r["out"].astype(F32).reshape(2, 256, nsh) for r in res.results]
    full = np.concatenate(per_core, axis=-1)  # [2, 256, N]
    return np.ascontiguousarray(full.transpose(0, 2, 1)).astype(F32)

